# revision 3
# baseline (speedup 1.0000x reference)
"""Trainium2 Bass kernel: single-head causal self-attention.

Problem: x:(8,2048,1024) f32, Wk/Wq/Wv:(1024,64) f32
  k,q,v = x@Wk, x@Wq, x@Wv ; S = q k^T / sqrt(64) causal-masked
  out = softmax(S) @ v  -> (8,2048,64) f32

Sharding: data-parallel over batch B=8 across the 8 NeuronCores (one batch
element per core).

Per-core design (all fp32):
  - Host passes x^T (C,T) per core so matmuls can contract over C on the
    partition dim with weights stationary.
  - Projections: psum(128,512) = [Wk|Wv]_c^T @ x^T_c accumulated over the 8
    C-tiles -> rows 0:64 = k^T, rows 64:128 = v^T (kept packed in one SBUF
    buffer "kvt"); q^T projected separately (M=64).
  - v^T -> v natural (keys on partitions) via PE transpose against an
    identity block at base-partition 64; a ones-column is appended (V') so
    the PV matmul also produces the softmax denominator.
  - Scores in transposed orientation: S^T_j = K_j Q^T (keys on partitions,
    queries on free dim) per 512-query block; causally-masked column ranges
    are simply never computed; diagonal 128x128 chunks get an additive
    -1e10 triangle mask (DVE) before exp.
  - exp(scale*S^T) on the scalar engine straight out of PSUM (no row-max
    subtraction: |scale*S| < ~4 so exp is well-conditioned).
  - out'^T = V'^T P^T accumulated in PSUM over key tiles; row 64 = sum(P).
  - Normalize: reciprocal of row 64, broadcast across 64 partitions via a
    rank-1 matmul with a ones vector, multiply. Result stays (64,2048)=out^T
    per core; the host transposes back when gathering.
"""

import os
import sys
from contextlib import ExitStack

import numpy as np

if "/opt/trn_rl_repo" not in sys.path:
    sys.path.insert(0, "/opt/trn_rl_repo")

import concourse.bacc as bacc
import concourse.bass as bass
import concourse.mybir as mybir
import concourse.tile as tile
from concourse.bass import ds
from concourse.bass_utils import run_bass_kernel_spmd
from concourse.masks import make_identity

F32 = mybir.dt.float32

B, T, C, H = 8, 2048, 1024, 64
P = 128           # partitions
CT = C // P       # 8 c-tiles
NBLK = 4          # query blocks of 512
QB = T // NBLK    # 512 queries per block
KT = T // P       # 16 key tiles
SCALE = H ** -0.5
NEG = -1.0e10


def build_bass():
    nc = bacc.Bacc("TRN2")

    xt = nc.dram_tensor("xt", (C, T), F32, kind="ExternalInput")
    wkv = nc.dram_tensor("wkv", (C, 2 * H), F32, kind="ExternalInput")
    wq = nc.dram_tensor("wq", (C, H), F32, kind="ExternalInput")
    out_t = nc.dram_tensor("out_t", (H, T), F32, kind="ExternalOutput")

    with ExitStack() as ctx:
        tc = ctx.enter_context(tile.TileContext(nc))
        const = ctx.enter_context(tc.tile_pool(name="const", bufs=1))
        ptp = ctx.enter_context(tc.tile_pool(name="ptp", bufs=3))
        sml = ctx.enter_context(tc.tile_pool(name="sml", bufs=2))
        psA = ctx.enter_context(tc.tile_pool(name="psA", bufs=4, space="PSUM"))
        psO = ctx.enter_context(tc.tile_pool(name="psO", bufs=2, space="PSUM"))
        psT = ctx.enter_context(tc.tile_pool(name="psT", bufs=2, space="PSUM"))

        # ---- persistent SBUF ----
        xt_sb = const.tile([P, CT, T], F32)        # x^T, c-tiled
        wkv_sb = const.tile([P, CT, 2 * H], F32)
        wq_sb = const.tile([P, CT, H], F32)
        kvt = const.tile([P, T], F32)              # rows 0:64 k^T, 64:128 v^T
        qt = const.tile([H, T], F32)               # q^T
        vsb = const.tile([P, KT, H + 1], F32)      # V' tiles (v | ones-col)
        outsb = const.tile([H, T], F32)            # out^T
        ident = const.tile([P, P], F32)
        cmask = const.tile([P, P], F32)            # additive causal triangle
        ones_t = const.tile([H + 1, H], F32)       # row 64 used as (1,64) of 1s

        # ---- constants ----
        nc.sync.dma_start(wkv_sb[:], wkv.rearrange("(c p) m -> p c m", p=P))
        nc.sync.dma_start(wq_sb[:], wq.rearrange("(c p) m -> p c m", p=P))
        make_identity(nc, ident)
        nc.gpsimd.memset(vsb[:], 1.0)              # ones-col survives v copies
        nc.gpsimd.memset(ones_t[:], 1.0)
        nc.gpsimd.memset(cmask[:], 0.0)
        # keep 0 where (query u) - (key s) >= 0, else -1e10  (mask s > u)
        nc.gpsimd.affine_select(
            out=cmask[:], in_=cmask[:],
            compare_op=mybir.AluOpType.is_ge, fill=NEG,
            base=0, pattern=[[1, P]], channel_multiplier=-1,
        )

        for c in range(CT):
            nc.sync.dma_start(xt_sb[:, c, :], xt[ds(c * P, P), :])

        def proj_chunk(g):
            sl = ds(g * QB, QB)
            pk = psA.tile([P, QB], F32, tag="mm")
            for c in range(CT):
                nc.tensor.matmul(pk[:], wkv_sb[:, c, :], xt_sb[:, c, sl],
                                 start=(c == 0), stop=(c == CT - 1))
            nc.scalar.copy(kvt[:, sl], pk[:])
            pq = psA.tile([H, QB], F32, tag="mm")
            for c in range(CT):
                nc.tensor.matmul(pq[:], wq_sb[:, c, :], xt_sb[:, c, sl],
                                 start=(c == 0), stop=(c == CT - 1))
            nc.vector.tensor_copy(qt[:, sl], pq[:])

        def v_nat(t):
            # (v^T chunk)^T -> v natural keys-on-partitions tile t
            ptr = psT.tile([P, H], F32, tag="tr")
            nc.tensor.matmul(ptr[:], kvt[H:P, ds(t * P, P)],
                             ident[H:P, H:P], start=True, stop=True)
            nc.vector.tensor_copy(vsb[:, t, 0:H], ptr[:])

        def attn_block(b):
            po = psO.tile([H + 1, QB], F32, tag="o")
            jmax = 4 * b + 3
            pts = []

            def pv(j):
                pt, c0 = pts[j]
                nc.tensor.matmul(po[:, c0:], vsb[:, j, :], pt[:, c0:],
                                 start=(j == 0), stop=(j == jmax))

            for j in range(jmax + 1):
                c0 = max(0, P * j - QB * b)
                ps = psA.tile([P, QB], F32, tag="mm")
                nc.tensor.matmul(ps[:, c0:], kvt[0:H, ds(j * P, P)],
                                 qt[:, ds(b * QB + c0, QB - c0)],
                                 start=True, stop=True)
                if P * j >= QB * b:  # diagonal tile: triangle mask
                    nc.vector.tensor_add(ps[:, c0:c0 + P], ps[:, c0:c0 + P],
                                         cmask[:])
                pt = ptp.tile([P, QB], F32, tag="pt")
                nc.scalar.activation(pt[:, c0:], ps[:, c0:],
                                     mybir.ActivationFunctionType.Exp,
                                     scale=SCALE)
                pts.append((pt, c0))
                if j > 0:
                    pv(j - 1)
            pv(jmax)

            # normalize: out^T = po[0:64] * (1/po[64]) broadcast
            sl = ds(b * QB, QB)
            rsb = sml.tile([H + 1, QB], F32, tag="rs")
            nc.vector.reciprocal(rsb[H:H + 1, :], po[H:H + 1, :])
            pb = psT.tile([H, QB], F32, tag="tr")
            nc.tensor.matmul(pb[:], ones_t[H:H + 1, :], rsb[H:H + 1, :],
                             start=True, stop=True)
            bc = sml.tile([H, QB], F32, tag="bc")
            nc.scalar.copy(bc[:], pb[:])
            nc.vector.tensor_mul(outsb[:, sl], po[0:H, :], bc[:])
            nc.sync.dma_start(out_t[:, sl], outsb[:, sl])

        for g in range(NBLK):
            proj_chunk(g)
            for t in range(4 * g, 4 * g + 4):
                v_nat(t)
            attn_block(g)

    nc.compile()
    return nc


_NC = None
LAST_EXEC_TIME_NS = None  # filled when BASS_TRACE=1 (read by test.py)
LAST_RESULT = None


def _get_nc():
    global _NC
    if _NC is None:
        _NC = build_bass()
    return _NC


def kernel(x, Wk, Wq, Wv):
    global LAST_EXEC_TIME_NS, LAST_RESULT
    x = np.ascontiguousarray(x, dtype=np.float32)
    wkv = np.ascontiguousarray(np.concatenate([Wk, Wv], axis=1), dtype=np.float32)
    wq = np.ascontiguousarray(Wq, dtype=np.float32)

    in_maps = []
    for b in range(B):
        in_maps.append({
            "xt": np.ascontiguousarray(x[b].T),
            "wkv": wkv,
            "wq": wq,
        })

    nc = _get_nc()
    res = run_bass_kernel_spmd(nc, in_maps, list(range(B)))
    LAST_EXEC_TIME_NS = res.exec_time_ns
    LAST_RESULT = res
    out = np.stack([np.ascontiguousarray(m["out_t"].T) for m in res.results])
    return out.astype(np.float32)


# revision 4
# speedup vs baseline: 1.9737x; 1.9737x over previous
"""Trainium2 Bass kernel: single-head causal self-attention.

Problem: x:(8,2048,1024) f32, Wk/Wq/Wv:(1024,64) f32
  k,q,v = x@Wk, x@Wq, x@Wv ; S = q k^T / sqrt(64) causal-masked
  out = softmax(S) @ v  -> (8,2048,64) f32

Sharding: data-parallel over batch B=8 across the 8 NeuronCores (one batch
element per core).

Per-core design (all fp32):
  - Host passes x^T (C,T) per core so matmuls can contract over C on the
    partition dim with weights stationary.
  - Projections: psum(128,512) = [Wk|Wv]_c^T @ x^T_c accumulated over the 8
    C-tiles -> rows 0:64 = k^T, rows 64:128 = v^T (kept packed in one SBUF
    buffer "kvt"); q^T projected separately (M=64).
  - v^T -> v natural (keys on partitions) via PE transpose against an
    identity block at base-partition 64; a ones-column is appended (V') so
    the PV matmul also produces the softmax denominator.
  - Scores in transposed orientation: S^T_j = K_j Q^T (keys on partitions,
    queries on free dim) per 512-query block; causally-masked column ranges
    are simply never computed; diagonal 128x128 chunks get an additive
    -1e10 triangle mask (DVE) before exp.
  - exp(scale*S^T) on the scalar engine straight out of PSUM (no row-max
    subtraction: |scale*S| < ~4 so exp is well-conditioned).
  - out'^T = V'^T P^T accumulated in PSUM over key tiles; row 64 = sum(P).
  - Normalize: reciprocal of row 64, broadcast across 64 partitions via a
    rank-1 matmul with a ones vector, multiply. Result stays (64,2048)=out^T
    per core; the host transposes back when gathering.
"""

import os
import sys
from contextlib import ExitStack

import numpy as np

if "/opt/trn_rl_repo" not in sys.path:
    sys.path.insert(0, "/opt/trn_rl_repo")

import concourse.bacc as bacc
import concourse.bass as bass
import concourse.mybir as mybir
import concourse.tile as tile
from concourse.bass import ds
from concourse.bass_utils import run_bass_kernel_spmd
from concourse.masks import make_identity

F32 = mybir.dt.float32
F16 = mybir.dt.float16

B, T, C, H = 8, 2048, 1024, 64
P = 128           # partitions
CT = C // P       # 8 c-tiles
NBLK = 4          # query blocks of 512
QB = T // NBLK    # 512 queries per block
KT = T // P       # 16 key tiles
SCALE = H ** -0.5
NEG = -1.0e10


def build_bass():
    nc = bacc.Bacc("TRN2")

    xt = nc.dram_tensor("xt", (C, T), F16, kind="ExternalInput")
    wkv = nc.dram_tensor("wkv", (C, 2 * H), F16, kind="ExternalInput")
    wq = nc.dram_tensor("wq", (C, H), F16, kind="ExternalInput")
    out_t = nc.dram_tensor("out_t", (H, T), F32, kind="ExternalOutput")

    with ExitStack() as ctx:
        tc = ctx.enter_context(tile.TileContext(nc))
        const = ctx.enter_context(tc.tile_pool(name="const", bufs=1))
        ptp = ctx.enter_context(tc.tile_pool(name="ptp", bufs=3))
        sml = ctx.enter_context(tc.tile_pool(name="sml", bufs=2))
        psA = ctx.enter_context(tc.tile_pool(name="psA", bufs=4, space="PSUM"))
        psO = ctx.enter_context(tc.tile_pool(name="psO", bufs=2, space="PSUM"))
        psT = ctx.enter_context(tc.tile_pool(name="psT", bufs=2, space="PSUM"))

        # ---- persistent SBUF ----
        xt_sb = const.tile([P, CT, T], F16)        # x^T, c-tiled
        wkv_sb = const.tile([P, CT, 2 * H], F16)
        wq_sb = const.tile([P, CT, H], F16)
        kvt = const.tile([P, T], F16)              # rows 0:64 k^T, 64:128 v^T
        qt = const.tile([H, T], F16)               # q^T
        vsb = const.tile([P, KT, H + 1], F16)      # V' tiles (v | ones-col)
        outsb = const.tile([H, T], F32)            # out^T
        ident = const.tile([P, P], F16)
        cmask = const.tile([P, P], F32)            # additive causal triangle
        ones_t = const.tile([H + 1, H], F16)       # row 64 used as (1,64) of 1s

        # ---- constants ----
        nc.sync.dma_start(wkv_sb[:], wkv.rearrange("(c p) m -> p c m", p=P))
        nc.sync.dma_start(wq_sb[:], wq.rearrange("(c p) m -> p c m", p=P))
        make_identity(nc, ident)
        nc.gpsimd.memset(vsb[:], 1.0)              # ones-col survives v copies
        nc.gpsimd.memset(ones_t[:], 1.0)
        nc.gpsimd.memset(cmask[:], 0.0)
        # keep 0 where (query u) - (key s) >= 0, else -1e10  (mask s > u)
        nc.gpsimd.affine_select(
            out=cmask[:], in_=cmask[:],
            compare_op=mybir.AluOpType.is_ge, fill=NEG,
            base=0, pattern=[[1, P]], channel_multiplier=-1,
        )

        for c in range(CT):
            nc.sync.dma_start(xt_sb[:, c, :], xt[ds(c * P, P), :])

        def proj_chunk(g):
            sl = ds(g * QB, QB)
            pk = psA.tile([P, QB], F32, tag="mm")
            for c in range(CT):
                nc.tensor.matmul(pk[:], wkv_sb[:, c, :], xt_sb[:, c, sl],
                                 start=(c == 0), stop=(c == CT - 1))
            nc.scalar.copy(kvt[:, sl], pk[:])
            pq = psA.tile([H, QB], F32, tag="mm")
            for c in range(CT):
                nc.tensor.matmul(pq[:], wq_sb[:, c, :], xt_sb[:, c, sl],
                                 start=(c == 0), stop=(c == CT - 1))
            nc.vector.tensor_copy(qt[:, sl], pq[:])

        def v_nat(t):
            # (v^T chunk)^T -> v natural keys-on-partitions tile t
            ptr = psT.tile([P, H], F32, tag="tr")
            nc.tensor.matmul(ptr[:], kvt[H:P, ds(t * P, P)],
                             ident[H:P, H:P], start=True, stop=True)
            nc.vector.tensor_copy(vsb[:, t, 0:H], ptr[:])

        def attn_block(b):
            po = psO.tile([H + 1, QB], F32, tag="o")
            jmax = 4 * b + 3
            pts = []

            def pv(j):
                pt, c0 = pts[j]
                nc.tensor.matmul(po[:, c0:], vsb[:, j, :], pt[:, c0:],
                                 start=(j == 0), stop=(j == jmax))

            for j in range(jmax + 1):
                c0 = max(0, P * j - QB * b)
                ps = psA.tile([P, QB], F32, tag="mm")
                nc.tensor.matmul(ps[:, c0:], kvt[0:H, ds(j * P, P)],
                                 qt[:, ds(b * QB + c0, QB - c0)],
                                 start=True, stop=True)
                if P * j >= QB * b:  # diagonal tile: triangle mask
                    nc.vector.tensor_add(ps[:, c0:c0 + P], ps[:, c0:c0 + P],
                                         cmask[:])
                pt = ptp.tile([P, QB], F16, tag="pt")
                nc.scalar.activation(pt[:, c0:], ps[:, c0:],
                                     mybir.ActivationFunctionType.Exp,
                                     scale=SCALE)
                pts.append((pt, c0))
                if j > 0:
                    pv(j - 1)
            pv(jmax)

            # normalize: out^T = po[0:64] * (1/po[64]) broadcast
            sl = ds(b * QB, QB)
            rsb = sml.tile([H + 1, QB], F16, tag="rs")
            lns = sml.tile([H + 1, QB], F32, tag="ln")
            nc.scalar.activation(lns[H:H + 1, :], po[H:H + 1, :],
                                 mybir.ActivationFunctionType.Ln)
            nc.scalar.activation(rsb[H:H + 1, :], lns[H:H + 1, :],
                                 mybir.ActivationFunctionType.Exp, scale=-1.0)
            pb = psT.tile([H, QB], F32, tag="tr")
            nc.tensor.matmul(pb[:], ones_t[H:H + 1, :], rsb[H:H + 1, :],
                             start=True, stop=True)
            bc = sml.tile([H, QB], F32, tag="bc")
            nc.scalar.copy(bc[:], pb[:])
            nc.vector.tensor_mul(outsb[:, sl], po[0:H, :], bc[:])
            nc.sync.dma_start(out_t[:, sl], outsb[:, sl])

        for g in range(NBLK):
            proj_chunk(g)
            for t in range(4 * g, 4 * g + 4):
                v_nat(t)
            attn_block(g)

    nc.compile()
    return nc


_NC = None
LAST_EXEC_TIME_NS = None  # filled when BASS_TRACE=1 (read by test.py)
LAST_RESULT = None


def _get_nc():
    global _NC
    if _NC is None:
        _NC = build_bass()
    return _NC


def kernel(x, Wk, Wq, Wv):
    global LAST_EXEC_TIME_NS, LAST_RESULT
    x = np.ascontiguousarray(x, dtype=np.float16)
    wkv = np.ascontiguousarray(np.concatenate([Wk, Wv], axis=1), dtype=np.float16)
    wq = np.ascontiguousarray(Wq, dtype=np.float16)

    in_maps = []
    for b in range(B):
        in_maps.append({
            "xt": np.ascontiguousarray(x[b].T),
            "wkv": wkv,
            "wq": wq,
        })

    nc = _get_nc()
    res = run_bass_kernel_spmd(nc, in_maps, list(range(B)))
    LAST_EXEC_TIME_NS = res.exec_time_ns
    LAST_RESULT = res
    out = np.stack([np.ascontiguousarray(m["out_t"].T) for m in res.results])
    return out.astype(np.float32)


# revision 5
# speedup vs baseline: 2.4175x; 1.2249x over previous
"""Trainium2 Bass kernel: single-head causal self-attention.

Problem: x:(8,2048,1024) f32, Wk/Wq/Wv:(1024,64) f32
  k,q,v = x@Wk, x@Wq, x@Wv ; S = q k^T / sqrt(64) causal-masked
  out = softmax(S) @ v  -> (8,2048,64) f32

Sharding: data-parallel over batch B=8 across the 8 NeuronCores (one batch
element per core).

Per-core design (fp16 matmul paths, fp32 PSUM accumulation):
  - Host passes x^T (C,T) fp16 per core so matmuls contract over C on the
    partition dim with weights stationary.
  - Warm-up: a few dummy matmuls run while x streams in so the PE HAM clock
    gate is already at 2.4 GHz when real work starts.
  - Projections: psum(128,512) = [Wk|Wv]_c^T @ x^T_c accumulated over the 8
    C-tiles -> rows 0:64 = k^T, rows 64:128 = v^T (packed buffer "kvt");
    q^T projected separately (M=64).
  - v^T -> v natural (keys on partitions) via PE transpose against an
    identity block at base-partition 64; a ones-column is appended (V') so
    the PV matmul also produces the softmax denominator.
  - Scores transposed: S^T_j = K_j Q^T (keys on partitions, queries free)
    per 512-query block; causally-dead column ranges are never computed.
  - exp(scale*S^T) on the scalar engine (its only activation -> one table
    load); no row-max subtraction (|scale*S| < ~4). Diagonal 128x128 chunks
    are masked AFTER exp by a multiplicative gpsimd affine_select (p=0).
  - out'^T = V'^T P^T accumulated in PSUM over key tiles; row 64 = sum(P).
  - Epilogue per 512-block: copy out'^T to SBUF (fp16), PE-transpose the
    four (65,128) chunks to natural (128,65), reciprocal of the denominator
    column (8-cycle, per-partition) and broadcast-multiply on DVE. Output is
    written natural (2048,64) fp32, so the host does no transpose.
"""

import os
import sys
from contextlib import ExitStack

import numpy as np

if "/opt/trn_rl_repo" not in sys.path:
    sys.path.insert(0, "/opt/trn_rl_repo")

import concourse.bacc as bacc
import concourse.bass as bass
import concourse.mybir as mybir
import concourse.tile as tile
from concourse.bass import ds
from concourse.bass_utils import run_bass_kernel_spmd
from concourse.masks import make_identity

F32 = mybir.dt.float32
F16 = mybir.dt.float16

B, T, C, H = 8, 2048, 1024, 64
P = 128           # partitions
CT = C // P       # 8 c-tiles
NBLK = 4          # query blocks of 512
QB = T // NBLK    # 512 queries per block
KT = T // P       # 16 key tiles
SCALE = H ** -0.5
N_WARM = 10


def build_bass():
    nc = bacc.Bacc("TRN2")

    xt = nc.dram_tensor("xt", (C, T), F16, kind="ExternalInput")
    wkv = nc.dram_tensor("wkv", (C, 2 * H), F16, kind="ExternalInput")
    wq = nc.dram_tensor("wq", (C, H), F16, kind="ExternalInput")
    out = nc.dram_tensor("out", (T, H), F32, kind="ExternalOutput")
    outv = out.rearrange("(i p) h -> p i h", p=P)   # (128, 16, 64) view

    with ExitStack() as ctx:
        tc = ctx.enter_context(tile.TileContext(nc))
        const = ctx.enter_context(tc.tile_pool(name="const", bufs=1))
        ptp = ctx.enter_context(tc.tile_pool(name="ptp", bufs=3))
        sml = ctx.enter_context(tc.tile_pool(name="sml", bufs=2))
        psA = ctx.enter_context(tc.tile_pool(name="psA", bufs=4, space="PSUM"))
        psO = ctx.enter_context(tc.tile_pool(name="psO", bufs=2, space="PSUM"))
        psT = ctx.enter_context(tc.tile_pool(name="psT", bufs=2, space="PSUM"))

        # ---- persistent SBUF ----
        xt_sb = const.tile([P, CT, T], F16)        # x^T, c-tiled
        wkv_sb = const.tile([P, CT, 2 * H], F16)
        wq_sb = const.tile([P, CT, H], F16)
        kvt = const.tile([P, T], F16)              # rows 0:64 k^T, 64:128 v^T
        qt = const.tile([H, T], F16)               # q^T
        vsb = const.tile([P, KT, H + 1], F16)      # V' tiles (v | ones-col)
        outn = const.tile([P, KT, H], F32)         # natural out tiles
        ident = const.tile([P, P], F16)
        wrm = const.tile([P, QB], F16)             # warm-up operand

        # ---- constants (no DMA deps -> issue immediately) ----
        make_identity(nc, ident)
        nc.gpsimd.memset(vsb[:], 1.0)              # ones-col survives v copies
        nc.gpsimd.memset(wrm[:], 0.25)

        nc.sync.dma_start(wkv_sb[:], wkv.rearrange("(c p) m -> p c m", p=P))
        nc.sync.dma_start(wq_sb[:], wq.rearrange("(c p) m -> p c m", p=P))
        for c in range(CT):
            nc.sync.dma_start(xt_sb[:, c, :], xt[ds(c * P, P), :])

        # ---- PE warm-up while x loads: keeps the HAM clock-gate at 2.4 GHz
        for w in range(N_WARM):
            pw = psT.tile([P, QB], F32, tag="tr")
            nc.tensor.matmul(pw[:], wrm[:, 0:P], wrm[:], start=True, stop=True)

        def proj_chunk(g):
            sl = ds(g * QB, QB)
            pk = psA.tile([P, QB], F32, tag="mm")
            for c in range(CT):
                nc.tensor.matmul(pk[:], wkv_sb[:, c, :], xt_sb[:, c, sl],
                                 start=(c == 0), stop=(c == CT - 1))
            nc.vector.tensor_copy(kvt[:, sl], pk[:])
            pq = psA.tile([H, QB], F32, tag="mm")
            for c in range(CT):
                nc.tensor.matmul(pq[:], wq_sb[:, c, :], xt_sb[:, c, sl],
                                 start=(c == 0), stop=(c == CT - 1))
            nc.vector.tensor_copy(qt[:, sl], pq[:])

        def v_nat(g):
            # 4 transposed v chunks into one psum, single batched copy out
            pn = psT.tile([P, 4, H], F32, tag="tr")
            for i in range(4):
                t = 4 * g + i
                nc.tensor.matmul(pn[:, i, :], kvt[H:P, ds(t * P, P)],
                                 ident[H:P, H:H + H], start=True, stop=True)
            nc.vector.tensor_copy(vsb[:, ds(4 * g, 4), 0:H], pn[:])

        def attn_block(b):
            po = psO.tile([H + 1, QB], F32, tag="o")
            jmax = 4 * b + 3
            pts = []

            def pv(j):
                pt, c0 = pts[j]
                nc.tensor.matmul(po[:, c0:], vsb[:, j, :], pt[:, c0:],
                                 start=(j == 0), stop=(j == jmax))

            for j in range(jmax + 1):
                c0 = max(0, P * j - QB * b)
                ps = psA.tile([P, QB], F32, tag="mm")
                nc.tensor.matmul(ps[:, c0:], kvt[0:H, ds(j * P, P)],
                                 qt[:, ds(b * QB + c0, QB - c0)],
                                 start=True, stop=True)
                pt = ptp.tile([P, QB], F16, tag="pt")
                nc.scalar.activation(pt[:, c0:], ps[:, c0:],
                                     mybir.ActivationFunctionType.Exp,
                                     scale=SCALE)
                if P * j >= QB * b:  # diagonal: zero p where key s > query u
                    nc.gpsimd.affine_select(
                        out=pt[:, c0:c0 + P], in_=pt[:, c0:c0 + P],
                        compare_op=mybir.AluOpType.is_ge, fill=0.0,
                        base=0, pattern=[[1, P]], channel_multiplier=-1,
                    )
                pts.append((pt, c0))
                if j > 0:
                    pv(j - 1)
            pv(jmax)

            # epilogue: transpose to natural, divide by denominator column
            posb = sml.tile([H + 1, QB], F16, tag="os")
            nc.vector.tensor_copy(posb[:], po[:])
            pn = psT.tile([P, 4, H + 1], F32, tag="tr")
            for i in range(4):
                nc.tensor.matmul(pn[:, i, :], posb[:, ds(i * P, P)],
                                 ident[0:H + 1, 0:H + 1], start=True, stop=True)
            onat = sml.tile([P, 4, H + 1], F32, tag="on")
            nc.vector.tensor_copy(onat[:], pn[:])
            rc = sml.tile([P, 4, 1], F32, tag="rc")
            nc.vector.reciprocal(rc[:], onat[:, :, H:H + 1])
            nc.vector.tensor_tensor(outn[:, ds(4 * b, 4), :],
                                    onat[:, :, 0:H],
                                    rc[:].to_broadcast((P, 4, H)),
                                    mybir.AluOpType.mult)
            nc.sync.dma_start(outv[:, ds(4 * b, 4), :], outn[:, ds(4 * b, 4), :])

        for g in range(NBLK):
            proj_chunk(g)
            v_nat(g)
            attn_block(g)

    nc.compile()
    return nc


_NC = None
LAST_EXEC_TIME_NS = None  # filled when BASS_TRACE=1 (read by test.py)
LAST_RESULT = None


def _get_nc():
    global _NC
    if _NC is None:
        _NC = build_bass()
    return _NC


def kernel(x, Wk, Wq, Wv):
    global LAST_EXEC_TIME_NS, LAST_RESULT
    x = np.ascontiguousarray(x, dtype=np.float16)
    wkv = np.ascontiguousarray(np.concatenate([Wk, Wv], axis=1), dtype=np.float16)
    wq = np.ascontiguousarray(Wq, dtype=np.float16)

    in_maps = []
    for b in range(B):
        in_maps.append({
            "xt": np.ascontiguousarray(x[b].T),
            "wkv": wkv,
            "wq": wq,
        })

    nc = _get_nc()
    res = run_bass_kernel_spmd(nc, in_maps, list(range(B)))
    LAST_EXEC_TIME_NS = res.exec_time_ns
    LAST_RESULT = res
    out = np.stack([np.ascontiguousarray(m["out"]) for m in res.results])
    return out.astype(np.float32)


# revision 6
# speedup vs baseline: 2.5761x; 1.0656x over previous
"""Trainium2 Bass kernel: single-head causal self-attention.

Problem: x:(8,2048,1024) f32, Wk/Wq/Wv:(1024,64) f32
  k,q,v = x@Wk, x@Wq, x@Wv ; S = q k^T / sqrt(64) causal-masked
  out = softmax(S) @ v  -> (8,2048,64) f32

Sharding: data-parallel over batch B=8 across the 8 NeuronCores (one batch
element per core).

Per-core design (fp16 matmul paths, fp32 PSUM accumulation):
  - Host passes x^T (C,T) fp16 per core so matmuls contract over C on the
    partition dim with weights stationary.
  - Warm-up: a few dummy matmuls run while x streams in so the PE HAM clock
    gate is already at 2.4 GHz when real work starts.
  - Projections: psum(128,512) = [Wk|Wv]_c^T @ x^T_c accumulated over the 8
    C-tiles -> rows 0:64 = k^T, rows 64:128 = v^T (packed buffer "kvt");
    q^T projected separately (M=64).
  - v^T -> v natural (keys on partitions) via PE transpose against an
    identity block at base-partition 64; a ones-column is appended (V') so
    the PV matmul also produces the softmax denominator.
  - Scores transposed: S^T_j = K_j Q^T (keys on partitions, queries free)
    per 512-query block; causally-dead column ranges are never computed.
  - exp(scale*S^T) on the scalar engine (its only activation -> one table
    load); no row-max subtraction (|scale*S| < ~4). Diagonal 128x128 chunks
    are masked AFTER exp by a multiplicative gpsimd affine_select (p=0).
  - out'^T = V'^T P^T accumulated in PSUM over key tiles; row 64 = sum(P).
  - Epilogue per 512-block: copy out'^T to SBUF (fp16), PE-transpose the
    four (65,128) chunks to natural (128,65), reciprocal of the denominator
    column (8-cycle, per-partition) and broadcast-multiply on DVE. Output is
    written natural (2048,64) fp32, so the host does no transpose.
"""

import os
import sys
from contextlib import ExitStack

import numpy as np

if "/opt/trn_rl_repo" not in sys.path:
    sys.path.insert(0, "/opt/trn_rl_repo")

import concourse.bacc as bacc
import concourse.bass as bass
import concourse.mybir as mybir
import concourse.tile as tile
from concourse.bass import ds
from concourse.bass_utils import run_bass_kernel_spmd
from concourse.masks import make_identity

F32 = mybir.dt.float32
F16 = mybir.dt.float16

B, T, C, H = 8, 2048, 1024, 64
P = 128           # partitions
CT = C // P       # 8 c-tiles
NBLK = 4          # query blocks of 512
QB = T // NBLK    # 512 queries per block
KT = T // P       # 16 key tiles
SCALE = H ** -0.5
N_WARM = 8


def build_bass():
    nc = bacc.Bacc("TRN2")

    xt = nc.dram_tensor("xt", (C, T), F16, kind="ExternalInput")
    wkv = nc.dram_tensor("wkv", (C, 2 * H), F16, kind="ExternalInput")
    wq = nc.dram_tensor("wq", (C, H), F16, kind="ExternalInput")
    out = nc.dram_tensor("out", (T, H), F32, kind="ExternalOutput")
    outv = out.rearrange("(i p) h -> p i h", p=P)   # (128, 16, 64) view

    with ExitStack() as ctx:
        tc = ctx.enter_context(tile.TileContext(nc))
        const = ctx.enter_context(tc.tile_pool(name="const", bufs=1))
        ptp = ctx.enter_context(tc.tile_pool(name="ptp", bufs=3))
        sml = ctx.enter_context(tc.tile_pool(name="sml", bufs=2))
        psA = ctx.enter_context(tc.tile_pool(name="psA", bufs=4, space="PSUM"))
        psO = ctx.enter_context(tc.tile_pool(name="psO", bufs=2, space="PSUM"))
        psT = ctx.enter_context(tc.tile_pool(name="psT", bufs=2, space="PSUM"))

        # ---- persistent SBUF ----
        xt_sb = const.tile([P, CT, T], F16)        # x^T, c-tiled
        wkv_sb = const.tile([P, CT, 2 * H], F16)
        wq_sb = const.tile([P, CT, H], F16)
        kvt = const.tile([P, T], F16)              # rows 0:64 k^T, 64:128 v^T
        qt = const.tile([H, T], F16)               # q^T
        vsb = const.tile([P, KT, H + 1], F16)      # V' tiles (v | ones-col)
        outn = const.tile([P, KT, H], F32)         # natural out tiles
        ident = const.tile([P, P], F16)
        wrm = const.tile([P, QB], F16)             # warm-up operand

        # ---- constants (no DMA deps -> issue immediately) ----
        nc.gpsimd.memset(wrm[:], 0.25)
        make_identity(nc, ident)
        nc.gpsimd.memset(vsb[:, :, H:H + 1], 1.0)  # V' ones-column

        nc.sync.dma_start(wkv_sb[:], wkv.rearrange("(c p) m -> p c m", p=P))
        nc.scalar.dma_start(wq_sb[:], wq.rearrange("(c p) m -> p c m", p=P))
        for c in range(CT):
            eng = nc.sync if c % 2 == 0 else nc.scalar
            eng.dma_start(xt_sb[:, c, :], xt[ds(c * P, P), :])

        # ---- PE warm-up while x loads: keeps the HAM clock-gate at 2.4 GHz
        warm = []
        for w in range(N_WARM):
            pw = psT.tile([P, QB], F32, tag="tr")
            nc.tensor.matmul(pw[:], wrm[:, 0:P], wrm[:], start=True, stop=True)

        def warm_mm():
            pw = psT.tile([P, QB], F32, tag="tr")
            nc.tensor.matmul(pw[:], wrm[:, 0:P], wrm[:], start=True, stop=True)

        def proj_chunk(g):
            sl = ds(g * QB, QB)
            pk = psA.tile([P, QB], F32, tag="mm")
            for c in range(CT):
                nc.tensor.matmul(pk[:], wkv_sb[:, c, :], xt_sb[:, c, sl],
                                 start=(c == 0), stop=(c == CT - 1))
                if g == 0:  # x still streaming in: keep the PE clock warm
                    warm_mm()
            nc.vector.tensor_copy(kvt[:, sl], pk[:])
            pq = psA.tile([H, QB], F32, tag="mm")
            for c in range(CT):
                nc.tensor.matmul(pq[:], wq_sb[:, c, :], xt_sb[:, c, sl],
                                 start=(c == 0), stop=(c == CT - 1))
                if g == 0:
                    warm_mm()
            nc.vector.tensor_copy(qt[:, sl], pq[:])

        def v_nat(g):
            # 4 transposed v chunks into one psum, single batched copy out
            pn = psT.tile([P, 4, H], F32, tag="tr")
            for i in range(4):
                t = 4 * g + i
                nc.tensor.matmul(pn[:, i, :], kvt[H:P, ds(t * P, P)],
                                 ident[H:P, H:H + H], start=True, stop=True)
            nc.vector.tensor_copy(vsb[:, ds(4 * g, 4), 0:H], pn[:])

        def attn_block(b):
            po = psO.tile([H + 1, QB], F32, tag="o")
            jmax = 4 * b + 3
            pts = []

            def pv(j):
                pt, c0 = pts[j]
                nc.tensor.matmul(po[:, c0:], vsb[:, j, :], pt[:, c0:],
                                 start=(j == 0), stop=(j == jmax))

            for j in range(jmax + 1):
                c0 = max(0, P * j - QB * b)
                ps = psA.tile([P, QB], F32, tag="mm")
                nc.tensor.matmul(ps[:, c0:], kvt[0:H, ds(j * P, P)],
                                 qt[:, ds(b * QB + c0, QB - c0)],
                                 start=True, stop=True)
                pt = ptp.tile([P, QB], F16, tag="pt")
                nc.scalar.activation(pt[:, c0:], ps[:, c0:],
                                     mybir.ActivationFunctionType.Exp,
                                     scale=SCALE)
                if P * j >= QB * b:  # diagonal: zero p where key s > query u
                    nc.gpsimd.affine_select(
                        out=pt[:, c0:c0 + P], in_=pt[:, c0:c0 + P],
                        compare_op=mybir.AluOpType.is_ge, fill=0.0,
                        base=0, pattern=[[1, P]], channel_multiplier=-1,
                    )
                pts.append((pt, c0))
                if j > 0:
                    pv(j - 1)
            pv(jmax)

            # epilogue: transpose to natural, divide by denominator column
            posb = sml.tile([H + 1, QB], F16, tag="os")
            nc.vector.tensor_copy(posb[:], po[:])
            pn = psT.tile([P, 4, H + 1], F32, tag="tr")
            for i in range(4):
                nc.tensor.matmul(pn[:, i, :], posb[:, ds(i * P, P)],
                                 ident[0:H + 1, 0:H + 1], start=True, stop=True)
            onat = sml.tile([P, 4, H + 1], F32, tag="on")
            nc.vector.tensor_copy(onat[:], pn[:])
            rc = sml.tile([P, 4, 1], F32, tag="rc")
            nc.vector.reciprocal(rc[:], onat[:, :, H:H + 1])
            nc.vector.tensor_tensor(outn[:, ds(4 * b, 4), :],
                                    onat[:, :, 0:H],
                                    rc[:].to_broadcast((P, 4, H)),
                                    mybir.AluOpType.mult)
            nc.sync.dma_start(outv[:, ds(4 * b, 4), :], outn[:, ds(4 * b, 4), :])

        for g in range(NBLK):
            proj_chunk(g)
            v_nat(g)
            attn_block(g)

    nc.compile()
    return nc


_NC = None
LAST_EXEC_TIME_NS = None  # filled when BASS_TRACE=1 (read by test.py)
LAST_RESULT = None


def _get_nc():
    global _NC
    if _NC is None:
        _NC = build_bass()
    return _NC


def kernel(x, Wk, Wq, Wv):
    global LAST_EXEC_TIME_NS, LAST_RESULT
    x = np.ascontiguousarray(x, dtype=np.float16)
    wkv = np.ascontiguousarray(np.concatenate([Wk, Wv], axis=1), dtype=np.float16)
    wq = np.ascontiguousarray(Wq, dtype=np.float16)

    in_maps = []
    for b in range(B):
        in_maps.append({
            "xt": np.ascontiguousarray(x[b].T),
            "wkv": wkv,
            "wq": wq,
        })

    nc = _get_nc()
    res = run_bass_kernel_spmd(nc, in_maps, list(range(B)))
    LAST_EXEC_TIME_NS = res.exec_time_ns
    LAST_RESULT = res
    out = np.stack([np.ascontiguousarray(m["out"]) for m in res.results])
    return out.astype(np.float32)
